# revision 55
# baseline (speedup 1.0000x reference)
"""Trainium2 Bass kernel for a dense transformer encoder layer.

Reference semantics (B=2, S=2048, D=1024, H=16, DH=64, HID=4096):
    q = einsum('bsd,hde->bhse', x, Wq) + bq          (q == k == v, source bug)
    prob = softmax(q @ q^T / sqrt(DH))
    attn = concat_heads(prob @ q)
    x1 = LN(x + attn);  ff = relu(x1 @ W1 + b1) @ W2 + b2;  out = LN(x1 + ff)

Sharding: 8 cores, core c -> batch b=c//4, token quarter t=c%4 (the host
rotates x so the core's 512 query tokens are rows 0:512; attention is
permutation-equivariant over keys).  Zero collectives, zero DRAM scratch.

All matmuls run in fp8e4, mostly with DoubleRow perf mode (two K=128
subtiles per instruction).  Weights are host-prescaled by powers of two so
fp8 stays in normal range; the scales fold into the exp() scale, the
softmax-denominator ones-column (8.0), and the FFN epilogue constants.
Scores contract over DH=64 only, so both operands broadcast their single
k-subtile (stride-0) and the doubling folds into the exp scale.  bq folds
into the q^T cast (per-partition scalar add) and into x_q on the host (the
softmax-weighted mean of a constant is that constant); b1 folds into the
relu; b2 rides a ones-row lhsT into the FFN2 accumulation.

prob@q ("wv") is token-major ([queries, DH+1] PSUM tiles, denominator in
column 64), so head outputs merge into the residual via one
scalar_tensor_tensor per (head, token-block) with no transposes.  FFN2 is
token-major too: ff lands in [tokens, D] PSUM and residual + LN2 run
straight out of PSUM.

The 512 queries are processed in four 128-query groups: each finished
group's LN1/FFN chain is drip-fed ("fills") into the next group's attention
stream, hidden under the exp() wall on the Activation engine — the critical
resource (~131072 exp elements per partition, ~150us floor).  qproj (q^T
for scores, q-natural fp8 for wv) likewise drips into group 0's stream; a
pending-queue defers wv/epilogue emission until the q-natural chunks they
read exist in the instruction stream (the PE queue is in-order, so a
consumer emitted before its producer would deadlock or read garbage).
Exp() overflow is prevented by a constant -2.5 exponent shift that cancels
in the softmax ratio.  The tail (last group's FFN) uses a 4-deep PSUM
rotation and keep-alive matmuls to hold the PE p-state through LN1.
"""

import os
import numpy as np

import concourse.bacc as bacc
import concourse.mybir as mybir
from concourse import tile
from concourse.bass_utils import run_bass_kernel_spmd

dt = mybir.dt
AF = mybir.ActivationFunctionType
ALU = mybir.AluOpType
DR = mybir.MatmulPerfMode.DoubleRow

B, S, D = 2, 2048, 1024
H, DH, HID = 16, 64, 4096
SQ = S // 4            # tokens per core (queries)
NG = 4                 # query groups
GQ = SQ // NG          # queries per group (128)
EPS = 1e-5
F32, BF16, F8 = dt.float32, dt.bfloat16, dt.float8e4

_BUILD_CACHE = {}


def _bc2(ap, n):
    """[P, n] slice -> [P, 2, n] stride-0 k-subtile broadcast."""
    return ap.rearrange("p (o m) -> p o m", o=1).to_broadcast(
        [ap.partition_size(), 2, n])


def _build(apply_affine: bool):
    if apply_affine in _BUILD_CACHE:
        return _BUILD_CACHE[apply_affine]

    nc = bacc.Bacc("TRN2", target_bir_lowering=False, debug=False,
                   num_devices=8)

    xT8_d = nc.dram_tensor("xT8", [D, S], F8, kind="ExternalInput").ap()
    wq8_d = nc.dram_tensor("wq8", [D, D], F8, kind="ExternalInput").ap()
    x_q = nc.dram_tensor("x_q", [SQ, D], F32, kind="ExternalInput").ap()
    w1_d = nc.dram_tensor("w1dr", [4096, D], F8, kind="ExternalInput").ap()
    bq_d = nc.dram_tensor("bq_r", [128, 8], F32, kind="ExternalInput").ap()
    b1_d = nc.dram_tensor("b1_r", [128, 32], F32, kind="ExternalInput").ap()
    w2_d = nc.dram_tensor("w2dr", [4096, D], F8, kind="ExternalInput").ap()
    w2b_d = nc.dram_tensor("w2b", [128, D], F8, kind="ExternalInput").ap()
    if apply_affine:
        g1d = nc.dram_tensor("g1d", [128, D], BF16, kind="ExternalInput").ap()
        be1d = nc.dram_tensor("be1d", [128, D], BF16,
                              kind="ExternalInput").ap()
        g2d = nc.dram_tensor("g2d", [128, D], BF16, kind="ExternalInput").ap()
        be2d = nc.dram_tensor("be2d", [128, D], BF16,
                              kind="ExternalInput").ap()
    out_q = nc.dram_tensor("out_q", [SQ, D], F32, kind="ExternalOutput").ap()
    KDBG = bool(int(os.environ.get("KDBG", "0")))
    if KDBG:
        dbg_qT0 = nc.dram_tensor("dbg_qT0", [128, S], F8,
                                 kind="ExternalOutput").ap()
        dbg_qnat = nc.dram_tensor("dbg_qnat", [128, 16 * H * 65], F8,
                                  kind="ExternalOutput").ap()
        dbg_y1 = nc.dram_tensor("dbg_y1", [4, 128, D], F32,
                                kind="ExternalOutput").ap()
        dbg_x1 = nc.dram_tensor("dbg_x1", [4, 128, D], BF16,
                                kind="ExternalOutput").ap()
        dbg_h18 = nc.dram_tensor("dbg_h18", [128, 2 * SQ], F8,
                                 kind="ExternalOutput").ap()
        dbg_x1T8 = nc.dram_tensor("dbg_x1T8", [128, 8 * SQ], F8,
                                  kind="ExternalOutput").ap()

    with tile.TileContext(nc) as tc:
        with (
            tc.tile_pool(name="const", bufs=1) as cpool,
            tc.tile_pool(name="qkv", bufs=1) as qpool,
            tc.tile_pool(name="ffw", bufs=1) as fwpool,
            tc.tile_pool(name="ln", bufs=2) as lnpool,
            tc.tile_pool(name="Epool", bufs=16) as Epool,
            tc.tile_pool(name="qps", bufs=2, space="PSUM") as qps,
            tc.tile_pool(name="scps", bufs=2, space="PSUM") as scps,
            tc.tile_pool(name="uvps", bufs=2, space="PSUM") as uvps,
        ):
            # ---- persistent SBUF ----
            qT8 = [qpool.tile([128, S], F8, tag=f"qT8_{p}", name=f"qT8_{p}")
                   for p in range(8)]
            qnat = qpool.tile([128, 16, H, 65], F8, tag="qnat")
            y1 = [qpool.tile([128, D], F32, tag=f"y1_{b}", name=f"y1_{b}")
                  for b in range(4)]
            x1bf = [qpool.tile([128, D], BF16, tag=f"x1_{b}", name=f"x1_{b}")
                    for b in range(4)]
            x1T8 = qpool.tile([128, 8, SQ], F8, tag="x1T8")
            xT8 = qpool.tile([128, 8, S], F8, tag="xT8")
            wq8 = qpool.tile([128, 8, D], F8, tag="wq8")
            bq_sb = qpool.tile([128, 8], F32, tag="bq")
            b1_sb = qpool.tile([128, 32], F32, tag="b1")
            h18 = [fwpool.tile([128, 2, SQ], F8, tag=f"h18_{j}",
                               name=f"h18_{j}") for j in range(16)]
            w18 = fwpool.tile([128, 32, 8, 128], F8, tag="w18")
            w28 = fwpool.tile([128, 16, 2, D], F8, tag="w28")
            w2b = fwpool.tile([128, D], F8, tag="w2b")
            h1one = cpool.tile([128, 128], F8, tag="h1one")
            eps_sb = cpool.tile([128, 1], F32, tag="eps")
            nc.vector.memset(eps_sb[:], EPS)
            negc = cpool.tile([128, 1], F32, tag="negc")
            nc.vector.memset(negc[:], -2.5)

            # constants: softmax-denominator column (8.0), FFN1 ones subtile,
            # FFN2 bias lhsT, bf16 identity for PE transposes
            nc.vector.memset(qnat[:, :, :, 64], 8.0)
            nc.vector.memset(h1one[:], 0.0)
            nc.vector.memset(h1one[0:1, :], 1.0)

            col_i = cpool.tile([128, 128], F32)
            nc.gpsimd.iota(col_i[:], [[1, 128]], channel_multiplier=0,
                           allow_small_or_imprecise_dtypes=True)
            row_i = cpool.tile([128, 1], F32)
            nc.gpsimd.iota(row_i[:], [[0, 1]], channel_multiplier=1,
                           allow_small_or_imprecise_dtypes=True)
            idn = cpool.tile([128, 128], BF16)
            nc.vector.tensor_scalar(idn[:], col_i[:], row_i[:, 0:1], None,
                                    ALU.is_equal)

            if apply_affine:
                g1_sb = cpool.tile([128, D], BF16, tag="g1")
                nc.scalar.dma_start(g1_sb[:], g1d[:])
                be1_sb = cpool.tile([128, D], BF16, tag="be1")
                nc.scalar.dma_start(be1_sb[:], be1d[:])
                g2_sb = cpool.tile([128, D], BF16, tag="g2")
                nc.scalar.dma_start(g2_sb[:], g2d[:])
                be2_sb = cpool.tile([128, D], BF16, tag="be2")
                nc.scalar.dma_start(be2_sb[:], be2d[:])

            # ---- input loads ----
            xT8_r = xT8_d.rearrange("(s p) t -> p s t", p=128)
            wq8_r = wq8_d.rearrange("(s p) m -> p s m", p=128)
            for i in range(4):
                eng = nc.sync if i % 2 == 0 else nc.scalar
                eng.dma_start(xT8[:, 2 * i:2 * i + 2, :],
                              xT8_r[:, 2 * i:2 * i + 2, :])
                eng2 = nc.scalar if i % 2 == 0 else nc.sync
                eng2.dma_start(wq8[:, 2 * i:2 * i + 2, :],
                               wq8_r[:, 2 * i:2 * i + 2, :])
            nc.sync.dma_start(bq_sb[:], bq_d[:])
            nc.sync.dma_start(b1_sb[:], b1_d[:])
            for b in range(4):
                eng = nc.sync if b % 2 == 0 else nc.scalar
                eng.dma_start(y1[b][:], x_q[b * 128:(b + 1) * 128, :])

            w1_r = w1_d.rearrange("(j p) x -> p j x", p=128)
            w18_f = w18.rearrange("p j s c -> p j (s c)")
            w2_r = w2_d.rearrange("(j s p) c -> p j s c", s=2, p=128)
            for i in range(4):
                eng = nc.sync if i % 2 == 0 else nc.scalar
                eng.dma_start(w18_f[:, 8 * i:8 * i + 8, :],
                              w1_r[:, 8 * i:8 * i + 8, :])
            for i in range(2):
                eng = nc.scalar if i % 2 == 0 else nc.sync
                eng.dma_start(w28[:, 8 * i:8 * i + 8], w2_r[:, 8 * i:8 * i + 8])
            nc.sync.dma_start(w2b[:], w2b_d[:])

            # ================= emitters =================
            def emit_qT(p, n):
                """q^T for pair p, quarter n -> qT8[p][:, n*512:(n+1)*512]."""
                ps = qps.tile([128, 512], F32, tag="qps", name=f"qps{p}_{n}")
                for t in range(4):
                    nc.tensor.matmul(
                        ps[:],
                        wq8[:, 2 * t:2 * t + 2, p * 128:(p + 1) * 128],
                        xT8[:, 2 * t:2 * t + 2, n * 512:(n + 1) * 512],
                        start=(t == 0), stop=(t == 3), perf_mode=DR)
                nc.vector.tensor_scalar_add(
                    qT8[p][:, n * 512:(n + 1) * 512], ps[:],
                    bq_sb[:, p:p + 1])

            def emit_qnat(c, hh):
                """natural q for token chunk c, head half hh (8 heads)."""
                ps = qps.tile([128, 512], F32, tag="qps", name=f"qn{c}_{hh}")
                for t in range(4):
                    nc.tensor.matmul(
                        ps[:],
                        xT8[:, 2 * t:2 * t + 2, c * 128:(c + 1) * 128],
                        wq8[:, 2 * t:2 * t + 2, hh * 512:(hh + 1) * 512],
                        start=(t == 0), stop=(t == 3), perf_mode=DR)
                dst = qnat[:, c, hh * 8:(hh + 1) * 8, 0:64]
                src = ps[:].rearrange("p (h e) -> p h e", h=8)
                if (c + hh) % 2 == 0:
                    nc.vector.tensor_copy(dst, src)
                else:
                    nc.scalar.copy(dst, src)

            def emit_wv_cg(uv, E, cg, h):
                for s2 in range(4):
                    nc.tensor.matmul(
                        uv[:, 0:65],
                        E[:, 2 * s2:2 * s2 + 2, :],
                        qnat[:, cg * 8 + 2 * s2:cg * 8 + 2 * s2 + 2,
                             h, :],
                        start=(cg == 0 and s2 == 0),
                        stop=(cg == 1 and s2 == 3),
                        perf_mode=DR)

            def emit_ln1(b):
                _layer_norm(nc, lnpool, y1[b], x1bf[b], eps_sb,
                            (g1_sb, be1_sb) if apply_affine else None,
                            pool_eng=(b < NG - 1))

            def emit_x1T(g):
                pst = qps.tile([128, 8, 128], BF16, tag="qps",
                               name=f"tp{g}")
                for k in range(8):
                    nc.tensor.transpose(
                        pst[:, k, :],
                        x1bf[g][:, k * 128:(k + 1) * 128], idn[:])
                nc.vector.tensor_copy(
                    x1T8[:, 0:8, g * 128:(g + 1) * 128], pst[:])

            def emit_ffn1(g, j):
                ps = qps.tile([128, 512], F32, tag="qps", name=f"f1{g}_{j}")
                for t in range(4):
                    nc.tensor.matmul(
                        ps[:, 0:GQ], w18[:, j, 2 * t:2 * t + 2, :],
                        x1T8[:, 2 * t:2 * t + 2, g * GQ:(g + 1) * GQ],
                        start=(t == 0), stop=(t == 3), perf_mode=DR)
                if g < NG - 1:
                    nc.vector.tensor_scalar(
                        h18[j // 2][:, j % 2, g * GQ:(g + 1) * GQ],
                        ps[:, 0:GQ], b1_sb[:, j:j + 1], 0.0,
                        ALU.add, ALU.max)
                else:
                    nc.scalar.activation(
                        h18[j // 2][:, j % 2, g * GQ:(g + 1) * GQ],
                        ps[:, 0:GQ], AF.Relu, bias=b1_sb[:, j:j + 1])

            def emit_ffn2(g):
                """FFN2 + residual + LN2 + store for group g's block."""
                b = g
                ps2 = [qps.tile([128, 512], F32, tag="qps",
                                name=f"ps2_{b}_{hf}") for hf in range(2)]
                for jp in range(16):
                    lhsT = h18[jp][:, :, b * 128:(b + 1) * 128]
                    for hf in range(2):
                        nc.tensor.matmul(
                            ps2[hf][:], lhsT,
                            w28[:, jp, :, hf * 512:(hf + 1) * 512],
                            start=(jp == 0), stop=False, perf_mode=DR)
                for hf in range(2):
                    nc.tensor.matmul(ps2[hf][:], h1one[:],
                                     w2b[:, hf * 512:(hf + 1) * 512],
                                     start=False, stop=True)
                y2 = lnpool.tile([128, D], F32, tag="y2", bufs=2,
                                 name=f"y2_{b}")
                for hf in range(2):
                    nc.vector.scalar_tensor_tensor(
                        y2[:, hf * 512:(hf + 1) * 512], ps2[hf][:],
                        0.0078125, x1bf[b][:, hf * 512:(hf + 1) * 512],
                        ALU.mult, ALU.add)
                x2 = lnpool.tile([128, D], F32, tag="x2", bufs=2,
                                 name=f"x2_{b}")
                _layer_norm(nc, lnpool, y2, x2, eps_sb,
                            (g2_sb, be2_sb) if apply_affine else None,
                            pool_eng=(b < NG - 1))
                nc.sync.dma_start(out_q[b * 128:(b + 1) * 128, :], x2[:])

            # ================= schedule =================
            fills = []          # (key, fn) closures drip-fed into the stream
            done_keys = set()

            def pop_fill(n=1):
                for _ in range(n):
                    if fills:
                        key, fn = fills.pop(0)
                        if key not in done_keys:
                            done_keys.add(key)
                            fn()

            def ensure(key):
                for i, (k, fn) in enumerate(list(fills)):
                    if k == key:
                        fills.pop(i)
                        if k not in done_keys:
                            done_keys.add(k)
                            fn()
                        return

            qnat_chunks = [0]   # fully-emitted qnat chunks (prefix count)

            def qnat_fill(c, hh):
                emit_qnat(c, hh)
                if hh == 1:
                    qnat_chunks[0] = c + 1

            # upfront: qT0 only; qnat + later pairs drip in as fills
            emit_qT(0, 0)
            emit_qT(0, 1)
            fills.append(("qT0_2", lambda: emit_qT(0, 2)))
            fills.append(("qT0_3", lambda: emit_qT(0, 3)))
            for c in range(6):
                for hh in range(2):
                    fills.append((f"qn{c}_{hh}",
                                  lambda c=c, hh=hh: qnat_fill(c, hh)))
            for n in range(4):
                fills.append((f"qT1_{n}", lambda n=n: emit_qT(1, n)))
            for c in range(6, 16):
                for hh in range(2):
                    fills.append((f"qn{c}_{hh}",
                                  lambda c=c, hh=hh: qnat_fill(c, hh)))

            # pending wv/epilogue items gated on emitted qnat chunks:
            # (chunks_needed, fn), flushed FIFO
            pend = []

            def flush_pend():
                while pend and qnat_chunks[0] >= pend[0][0]:
                    pend.pop(0)[1]()

            def emit_epi(g, h, uv):
                rct = lnpool.tile([128, 1], F32, tag="rct", bufs=2,
                                  name=f"rct{g}_{h}")
                nc.vector.reciprocal(rct[:], uv[:, 64:65])
                nc.vector.scalar_tensor_tensor(
                    y1[g][:, h * 64:(h + 1) * 64],
                    uv[:, 0:64], rct[:, 0:1],
                    y1[g][:, h * 64:(h + 1) * 64],
                    ALU.mult, ALU.add)

            rnd = 0
            # schedule: heads 0-1 of ALL groups first (they only need qT
            # pair 0, so the exp wall has guaranteed-ready work while the
            # qproj fills drain), then group-major for the rest
            sched = [(g, h) for h in (0, 1) for g in range(NG)]
            sched += [(g, h) for g in range(NG) for h in range(2, H)]
            heads_done = {g: 0 for g in range(NG)}
            for g, h in sched:
                qslc = slice(g * GQ, (g + 1) * GQ)
                p, half = h // 2, h % 2
                ensure(f"qT{p}_0")
                ensure(f"qT{p}_1")
                ensure(f"qT{p}_2")
                ensure(f"qT{p}_3")
                rows = slice(half * 64, half * 64 + 64)
                uv = uvps.tile([128, 512], F32, tag="uv",
                               name=f"uv{g}_{h}")
                Es = []
                for cg in range(2):
                    sc = scps.tile([128, 8, 128], F32, tag="sc",
                                   name=f"sc{g}_{h}_{cg}")
                    for kc in range(8):
                        c = cg * 8 + kc
                        nc.tensor.matmul(
                            sc[:, kc, :],
                            _bc2(qT8[p][rows, c * 128:(c + 1) * 128],
                                 128),
                            _bc2(qT8[p][rows, qslc], 128),
                            start=True, stop=True, perf_mode=DR)
                    E = Epool.tile([128, 8, 128], F8, tag="E",
                                   name=f"E{g}_{h}_{cg}")
                    nc.scalar.activation(E[:], sc[:], AF.Exp,
                                         scale=0.0009765625,
                                         bias=negc[:, 0:1])
                    Es.append(E)
                    pend.append((8 * (cg + 1),
                                 lambda uv=uv, E=E, cg=cg, h=h:
                                 emit_wv_cg(uv, E, cg, h)))
                    pop_fill(3 if rnd < 16 else (2 if rnd < 48 else 1))
                    flush_pend()
                    rnd += 1
                pend.append((16, lambda g=g, h=h, uv=uv:
                             emit_epi(g, h, uv)))
                flush_pend()
                if h % 2 == 1 and h // 2 + 2 < 8 and g == 0:
                    pn = h // 2 + 2
                    for n in range(4):
                        fills.append((f"qT{pn}_{n}",
                                      lambda pn=pn, n=n:
                                      emit_qT(pn, n)))
                heads_done[g] += 1
                if heads_done[g] == H and g < NG - 1:
                    fills.append((f"ln1_{g}", lambda g=g: emit_ln1(g)))
                    fills.append((f"x1T_{g}", lambda g=g: emit_x1T(g)))
                    for j in range(32):
                        fills.append((f"f1_{g}_{j}",
                                      lambda g=g, j=j: emit_ffn1(g, j)))
                    fills.append((f"f2_{g}", lambda g=g: emit_ffn2(g)))

            flush_pend()
            assert not pend
            while fills:
                pop_fill()

            # ---- last group tail ----
            emit_ln1(NG - 1)
            gl = NG - 1
            # p-state keep-alive: harmless DR matmuls into a scratch bank
            # while the LN1 chain runs on DVE, so the tail FFN starts at
            # full clock
            kps = qps.tile([128, 512], F32, tag="qps", name="keepalive")
            for i in range(28):
                nc.tensor.matmul(
                    kps[:], wq8[:, 0:2, 0:128], xT8[:, 0:2, 0:512],
                    start=(i == 0), stop=(i == 27), perf_mode=DR)
            emit_x1T(gl)
            ps2t = [uvps.tile([128, 512], F32, tag="uv", bufs=2,
                              name=f"ps2t_{hf}") for hf in range(2)]

            def tail_ffn1(j):
                # 4-deep psum rotation (qps slots + idle sc slots) to cover
                # the matmul->relu->reuse semaphore latency
                if j % 2 == 0:
                    ps = qps.tile([128, 512], F32, tag="qps",
                                  name=f"t1_{j}")[:]
                else:
                    ps = scps.tile([128, 8, 128], F32, tag="sc",
                                   name=f"t1_{j}").rearrange(
                        "p a b -> p (a b)")[:, 0:512]
                for t in range(4):
                    nc.tensor.matmul(
                        ps[:, 0:GQ], w18[:, j, 2 * t:2 * t + 2, :],
                        x1T8[:, 2 * t:2 * t + 2, gl * GQ:(gl + 1) * GQ],
                        start=(t == 0), stop=(t == 3), perf_mode=DR)
                if j % 2 == 0:
                    nc.scalar.activation(
                        h18[j // 2][:, j % 2, gl * GQ:(gl + 1) * GQ],
                        ps[:, 0:GQ], AF.Relu, bias=b1_sb[:, j:j + 1])
                else:
                    nc.vector.tensor_scalar(
                        h18[j // 2][:, j % 2, gl * GQ:(gl + 1) * GQ],
                        ps[:, 0:GQ], b1_sb[:, j:j + 1], 0.0,
                        ALU.add, ALU.max)

            for jp in range(16):
                tail_ffn1(2 * jp)
                tail_ffn1(2 * jp + 1)
                lhsT = h18[jp][:, :, gl * 128:(gl + 1) * 128]
                for hf in range(2):
                    nc.tensor.matmul(
                        ps2t[hf][:], lhsT,
                        w28[:, jp, :, hf * 512:(hf + 1) * 512],
                        start=(jp == 0), stop=False, perf_mode=DR)
            for hf in range(2):
                nc.tensor.matmul(ps2t[hf][:], h1one[:],
                                 w2b[:, hf * 512:(hf + 1) * 512],
                                 start=False, stop=True)
            y2 = lnpool.tile([128, D], F32, tag="y2", bufs=2, name="y2_t")
            for hf in range(2):
                nc.vector.scalar_tensor_tensor(
                    y2[:, hf * 512:(hf + 1) * 512], ps2t[hf][:],
                    0.0078125, x1bf[gl][:, hf * 512:(hf + 1) * 512],
                    ALU.mult, ALU.add)
            x2 = lnpool.tile([128, D], F32, tag="x2", bufs=2, name="x2_t")
            _layer_norm(nc, lnpool, y2, x2, eps_sb,
                        (g2_sb, be2_sb) if apply_affine else None,
                        pool_eng=False)
            nc.sync.dma_start(out_q[gl * 128:(gl + 1) * 128, :], x2[:])

            if KDBG:
                nc.sync.dma_start(dbg_qT0[:], qT8[0][:])
                nc.sync.dma_start(
                    dbg_qnat[:], qnat.rearrange("p a b c -> p (a b c)"))
                for b in range(4):
                    nc.sync.dma_start(dbg_y1[b], y1[b][:])
                    nc.sync.dma_start(dbg_x1[b], x1bf[b][:])
                nc.sync.dma_start(dbg_h18[:],
                                  h18[0].rearrange("p a b -> p (a b)"))
                nc.sync.dma_start(dbg_x1T8[:],
                                  x1T8.rearrange("p a b -> p (a b)"))

    nc.compile()
    _BUILD_CACHE[apply_affine] = nc
    return nc


def _layer_norm(nc, pool, y, out, eps_sb, affine, pool_eng=False):
    """out = (y - mean(y)) * rsqrt(var(y) + EPS) [* g + b] over free dim D.

    var = E[y^2] - E[y]^2 (no centering pass); smalls are [128,1].
    pool_eng=True runs the big passes on GpSimd (SBUF-only) to stay off the
    DVE during the attention exp() wall; False uses DVE + one ACT pass.
    """
    s1 = pool.tile([128, 1], F32, tag="ln_s1")
    nc.vector.reduce_sum(s1[:], y[:], axis=mybir.AxisListType.X)
    scr = pool.tile([128, D], F32, tag="ln_scr", bufs=2)
    ss = pool.tile([128, 1], F32, tag="ln_ss")
    if pool_eng:
        nc.gpsimd.tensor_mul(scr[:], y[:], y[:])
        nc.vector.reduce_sum(ss[:], scr[:], axis=mybir.AxisListType.X)
    else:
        nc.scalar.activation(scr[:], y[:], AF.Square, accum_out=ss[:])
    mean = pool.tile([128, 1], F32, tag="ln_mean")
    nc.vector.tensor_scalar_mul(mean[:], s1[:], 1.0 / D)
    m2 = pool.tile([128, 1], F32, tag="ln_m2")
    nc.vector.tensor_mul(m2[:], mean[:], mean[:])
    var = pool.tile([128, 1], F32, tag="ln_var")
    nc.vector.scalar_tensor_tensor(var[:], ss[:], 1.0 / D, m2[:],
                                   ALU.mult, ALU.subtract)
    std = pool.tile([128, 1], F32, tag="ln_std")
    nc.scalar.activation(std[:], var[:], AF.Sqrt, bias=eps_sb[:, 0:1])
    rstd = pool.tile([128, 1], F32, tag="ln_rstd")
    nc.vector.reciprocal(rstd[:], std[:])
    nc.vector.tensor_scalar(out[:], y[:], mean[:, 0:1], rstd[:, 0:1],
                            ALU.subtract, ALU.mult)
    if affine is not None:
        g_sb, b_sb = affine
        nc.vector.tensor_mul(out[:], out[:], g_sb[:])
        nc.vector.tensor_add(out[:], out[:], b_sb[:])


def kernel(x, Wq, bq, ln1_g, ln1_b, W1, b1, W2, b2, ln2_g, ln2_b):
    x = np.asarray(x, np.float32)
    f8 = dt.np(F8)
    trivial = (np.all(ln1_g == 1) and np.all(ln1_b == 0)
               and np.all(ln2_g == 1) and np.all(ln2_b == 0))
    nc = _build(apply_affine=not trivial)

    wq_flat = np.asarray(Wq, np.float32).transpose(1, 0, 2).reshape(D, D)
    bq_flat = 8.0 * np.asarray(bq, np.float32).reshape(D)
    wq8 = (8.0 * wq_flat).astype(f8)
    bq_r = np.ascontiguousarray(bq_flat.reshape(8, 128).T)
    b1_r = np.ascontiguousarray(
        (8.0 * np.asarray(b1, np.float32)).reshape(32, 128).T)

    # w1dr: [4096, 1024]; rows j*128+p, cols s*128+c = 8*W1[s*128+p, j*128+c]
    w1s = (8.0 * np.asarray(W1, np.float32)).reshape(8, 128, 32, 128)
    w1dr = np.ascontiguousarray(
        w1s.transpose(2, 1, 0, 3).reshape(4096, D)).astype(f8)

    # w2dr: [4096, 1024]; rows (jp*2+s)*128+p = 16*W2;  w2b row0 = 128*b2
    w2dr = (16.0 * np.asarray(W2, np.float32)).astype(f8)
    w2b = np.zeros((128, D), np.float32)
    w2b[0] = 128.0 * np.asarray(b2, np.float32)
    w2b = w2b.astype(f8)

    base = {"wq8": wq8, "w1dr": w1dr, "w2dr": w2dr, "w2b": w2b,
            "bq_r": bq_r, "b1_r": b1_r}
    if not trivial:
        bf = dt.np(BF16)
        for name, v in (("g1d", ln1_g), ("be1d", ln1_b),
                        ("g2d", ln2_g), ("be2d", ln2_b)):
            base[name] = np.ascontiguousarray(
                np.broadcast_to(np.asarray(v, np.float32),
                                (128, D))).astype(bf)

    in_maps = []
    for c in range(8):
        b, t = divmod(c, 4)
        xb = np.concatenate([x[b, t * SQ:], x[b, :t * SQ]], axis=0)
        in_maps.append({
            **base,
            "xT8": np.ascontiguousarray(xb.T).astype(f8),
            "x_q": np.ascontiguousarray(
                x[b, t * SQ:(t + 1) * SQ] + bq_flat[None, :] / 8.0),
        })

    res = run_bass_kernel_spmd(nc, in_maps, core_ids=list(range(8)))
    out = np.empty((B, S, D), np.float32)
    for c in range(8):
        b, t = divmod(c, 4)
        out[b, t * SQ:(t + 1) * SQ] = res.results[c]["out_q"]
    return out
